# revision 3
# baseline (speedup 1.0000x reference)
"""Trainium2 Bass kernel for the CPC (wav2vec-style contrastive) module.

Strategy (data-parallel over batch, 2 batches per core on 8 cores):
  1. proj[d, s, t] = sum_c W[c, d, s] * x[c, t]    (PE matmuls, fp16 in / fp32 acc)
  2. Gather target columns of y per (timestep u, copy n) with dma_gather
     (copy 0 = positive column u itself, copies 1..10 = sampled negatives,
      copy 11 = dummy padding so 12 copies align nicely with 128).
  3. Contrastive logits via block-diagonal matmuls:
        out[(u, n), (i, u')] = sum_d G[d, (u, n)] * proj[d, i, u' - 1 - i]
     Only the u == u' entries are wanted; host extracts that diagonal.
Everything heavy (both einsums and the negative gather) runs on-device.
"""

import numpy as np

import concourse.bass as bass
import concourse.mybir as mybir
import concourse.tile as tile
from concourse import bacc
from concourse.bass_utils import run_bass_kernel_spmd

# Problem constants (hardcoded per contest rules).
B, C, T = 16, 768, 1024
S = 12              # prediction steps
NNEG = 10
COPIES = NNEG + 1   # positive + negatives
CP = 12             # padded copies per timestep (extra dummy)
OFFSET = 1
NCORES = 8
NB = B // NCORES    # batches per core
CH = C // 128       # channel chunks
NQ = 4              # t-quarters
TQ = T // NQ        # 256
HIST = 13           # history columns kept per quarter (need t-1..t-12 plus 1 slack)
PW = TQ + HIST      # 269 = per-(chunk, step) block width in the proj buffer
GT_U = 32           # timesteps per gather tile
NGT = T // GT_U     # 32 gather tiles per batch
NI = GT_U * CP      # 384 gather indices per tile (must be % 128)
ST_U = 8            # timesteps per contrastive subtile
NSS = GT_U // ST_U  # 4 subtiles per gather tile
M_C = ST_U * CP     # 96 psum partitions per contrastive subtile
N_C = S * ST_U      # 96 moving columns per contrastive subtile

F16 = mybir.dt.float16
F32 = mybir.dt.float32
I16 = mybir.dt.int16

_CACHE = {}


def _build(repeat=1):
    nc = bacc.Bacc(None, target_bir_lowering=False)
    x_d = nc.dram_tensor("x2", [NB, C, T], F16, kind="ExternalInput")
    w_d = nc.dram_tensor("w", [S, C, C], F16, kind="ExternalInput")
    yt_d = nc.dram_tensor("yt2", [NB, T, C], F16, kind="ExternalInput")
    idx_d = nc.dram_tensor("idx2", [NB, NGT, 128, NI // 16], I16,
                           kind="ExternalInput")
    praw_d = nc.dram_tensor("praw", [NB, NGT, M_C, NSS * N_C], F16,
                            kind="ExternalOutput")

    with tile.TileContext(nc) as tc:
        with (
            tc.tile_pool(name="wpool", bufs=1) as wpool,
            tc.tile_pool(name="xpool", bufs=1) as xpool,
            tc.tile_pool(name="projpool", bufs=1) as projpool,
            tc.tile_pool(name="gpool", bufs=3) as gpool,
            tc.tile_pool(name="idxpool", bufs=1) as idxpool,
            tc.tile_pool(name="stpool", bufs=3) as stpool,
            tc.tile_pool(name="ppsum", bufs=3, space="PSUM") as ppsum,
            tc.tile_pool(name="cpsum", bufs=3, space="PSUM") as cpsum,
        ):
            # Weights resident for the whole kernel: [p, s, chunk, d]
            w_sb = wpool.tile([128, S, CH, C], F16)
            nc.sync.dma_start(w_sb[:], w_d.rearrange("s (a p) d -> p s a d", p=128))

            def body(_i=None):
                _core_body(nc, tc, x_d, yt_d, idx_d, praw_d, w_sb,
                           xpool, projpool, gpool, idxpool, stpool,
                           ppsum, cpsum)

            if repeat == 1:
                body()
            else:
                with tc.For_i(0, repeat, 1):
                    body()

    nc.compile()
    return nc


def _core_body(nc, tc, x_d, yt_d, idx_d, praw_d, w_sb,
               xpool, projpool, gpool, idxpool, stpool, ppsum, cpsum):
    if True:
        if True:
            for b in range(NB):
                x_sb = xpool.tile([128, CH, T], F16)
                nc.sync.dma_start(x_sb[:],
                                  x_d[b].rearrange("(a p) t -> p a t", p=128))
                idx_sb = idxpool.tile([128, NGT, NI // 16], I16)
                nc.sync.dma_start(idx_sb[:],
                                  idx_d[b].rearrange("g p c -> p g c"))

                # proj quarter buffer: free layout (chunk, step, col),
                # col = HIST + (t - TQ*q); cols [0, HIST) hold history.
                pq = projpool.tile([128, CH, S, PW], F16)
                pq_ap = pq[:]
                PQF = CH * S * PW  # free elements per partition

                def pq_custom(extra_off, dims):
                    return bass.AP(pq_ap.tensor, pq_ap.offset + extra_off,
                                   [[PQF, 128]] + dims)

                # zero history pads (t < 0 for q=0); garbage there is
                # harmless numerically but keep sim/NaN clean.
                nc.any.memset(
                    pq_custom(0, [[S * PW, CH], [PW, S], [1, HIST]]), 0.0)

                for q in range(NQ):
                    if q > 0:
                        # carry last HIST columns into the history slots
                        nc.vector.tensor_copy(
                            pq_custom(0, [[S * PW, CH], [PW, S], [1, HIST]]),
                            pq_custom(TQ, [[S * PW, CH], [PW, S], [1, HIST]]),
                        )
                    # ---- projection for this quarter ----
                    for m in range(CH):
                        for sp in range(S // 2):
                            ps = ppsum.tile([128, 2 * TQ], F32)
                            for sh in range(2):
                                s = 2 * sp + sh
                                for k in range(CH):
                                    nc.tensor.matmul(
                                        ps[:, TQ * sh:TQ * (sh + 1)],
                                        w_sb[:, s, k, 128 * m:128 * (m + 1)],
                                        x_sb[:, k, TQ * q:TQ * (q + 1)],
                                        start=(k == 0), stop=(k == CH - 1),
                                    )
                            dst = pq_custom(
                                m * (S * PW) + (2 * sp) * PW + HIST,
                                [[PW, 2], [1, TQ]])
                            nc.vector.tensor_copy(
                                dst, ps[:].rearrange("p (a t) -> p a t", a=2))

                    # ---- contrastive logits for this quarter ----
                    for g in range(NGT // NQ):
                        gt = (NGT // NQ) * q + g
                        G = gpool.tile([128, CH, NI], F16)
                        nc.gpsimd.dma_gather(
                            G[:], yt_d[b], idx_sb[:, gt, :],
                            NI, NI, C, transpose=True)
                        cps = cpsum.tile([128, NSS * N_C], F32)
                        for ss in range(NSS):
                            u_q = g * GT_U + ss * ST_U  # u offset in quarter
                            for k in range(CH):
                                rhs = pq_custom(
                                    k * (S * PW) + (HIST - 1) + u_q,
                                    [[PW - 1, S], [1, ST_U]])
                                nc.tensor.matmul(
                                    cps[0:M_C, N_C * ss:N_C * (ss + 1)],
                                    G[:, k, M_C * ss:M_C * (ss + 1)],
                                    rhs,
                                    start=(k == 0), stop=(k == CH - 1),
                                )
                        stg = stpool.tile([M_C, NSS * N_C], F16)
                        nc.vector.tensor_copy(stg[:], cps[0:M_C, :])
                        nc.sync.dma_start(praw_d[b, gt], stg[:])


def _get_nc():
    if "nc" not in _CACHE:
        _CACHE["nc"] = _build()
    return _CACHE["nc"]


def _prep_inputs(x, y, W, neg_idxs):
    x16 = x.astype(np.float16)                                     # [B, C, T]
    w16 = np.ascontiguousarray(W.transpose(2, 0, 1).astype(np.float16))  # [S,C,C]
    yt16 = np.ascontiguousarray(np.swapaxes(y, 1, 2).astype(np.float16))  # [B,T,C]

    nl = (neg_idxs.astype(np.int64)
          - (np.arange(B, dtype=np.int64)[:, None] * T))           # [B, NNEG*T]
    nl = nl.reshape(B, NNEG, T)
    idxf = np.zeros((B, T, CP), np.int16)
    idxf[:, :, 0] = np.arange(T, dtype=np.int16)[None, :]
    idxf[:, :, 1:COPIES] = nl.transpose(0, 2, 1)
    # copy CP-1 stays 0 (dummy)
    flat = idxf.reshape(B, NGT, NI)
    wrap = flat.reshape(B, NGT, NI // 16, 16).transpose(0, 1, 3, 2)  # [B,NGT,16,NI/16]
    idx_sb = np.ascontiguousarray(np.tile(wrap, (1, 1, 8, 1)))       # [B,NGT,128,NI/16]
    return x16, w16, yt16, idx_sb, nl


def kernel(x, y, W, b, neg_idxs, _trace=False):
    x = np.asarray(x, np.float32)
    y = np.asarray(y, np.float32)
    W = np.asarray(W, np.float32)
    b = np.asarray(b, np.float32)
    neg_idxs = np.asarray(neg_idxs)

    nc = _get_nc()
    x16, w16, yt16, idx_sb, nl = _prep_inputs(x, y, W, neg_idxs)

    in_maps = []
    for ci in range(NCORES):
        sl = slice(NB * ci, NB * (ci + 1))
        in_maps.append({
            "x2": np.ascontiguousarray(x16[sl]),
            "w": w16,
            "yt2": np.ascontiguousarray(yt16[sl]),
            "idx2": np.ascontiguousarray(idx_sb[sl]),
        })

    res = run_bass_kernel_spmd(nc, in_maps, core_ids=list(range(NCORES)),
                               trace=_trace)
    _CACHE["last_result"] = res

    # ---- host-side diagonal extraction + assembly ----
    P = np.empty((B, T, CP, S), np.float32)
    ar = np.arange(ST_U)
    for ci in range(NCORES):
        pr = res.results[ci]["praw"].astype(np.float32)  # [NB, NGT, 96, 384]
        R = pr.reshape(NB, NGT, ST_U, CP, NSS, S, ST_U)
        # diagonal over u_loc (axes 2 and 6) -> [ST_U, NB, NGT, CP, NSS, S]
        D = R[:, :, ar, :, :, :, ar]
        # -> [NB, NGT, NSS, ST_U, CP, S] -> [NB, T, CP, S]
        P[NB * ci:NB * (ci + 1)] = (
            D.transpose(1, 2, 4, 0, 3, 5).reshape(NB, T, CP, S))

    # exact bias correction: preds += dot(bias, target_column)
    if np.any(b):
        dby = np.einsum("d,bdt->bt", b, y)               # [B, T]
        corr = np.empty((B, T, COPIES), np.float32)
        corr[:, :, 0] = dby
        bidx = np.arange(B)[:, None, None]
        corr[:, :, 1:] = dby[bidx, nl.transpose(0, 2, 1)]
        P[:, :, :COPIES, :] += corr[:, :, :, None]

    preds = []
    for i in range(S):
        off = i + OFFSET
        blk = P[:, off:T, :COPIES, i]          # [B, T-off, COPIES]
        preds.append(np.ascontiguousarray(blk.transpose(1, 0, 2)).reshape(-1, COPIES))
    predictions = np.concatenate(preds, axis=0)
    labels = np.zeros((predictions.shape[0],), np.int32)
    return predictions, labels


# revision 9
# speedup vs baseline: 682.2832x; 682.2832x over previous
"""Trainium2 Bass kernel for the CPC (wav2vec-style contrastive) module.

Strategy (data-parallel over batch, 2 batches per core on 8 cores):
  1. proj[d, s, t] = sum_c W[c, d, s] * x[c, t]    (PE matmuls, fp16 in / fp32 acc)
  2. Gather target columns of y per (timestep u, copy n) with dma_gather
     (copy 0 = positive column u itself, copies 1..10 = sampled negatives,
      copy 11 = dummy padding so 12 copies align nicely with 128).
  3. Contrastive logits via block-diagonal matmuls:
        out[(u, n), (i, u')] = sum_d G[d, (u, n)] * proj[d, i, u' - 1 - i]
     Only the u == u' entries are wanted; host extracts that diagonal.
Everything heavy (both einsums and the negative gather) runs on-device.
"""

import numpy as np

import concourse.bass as bass
import concourse.mybir as mybir
import concourse.tile as tile
from concourse import bacc
from concourse.bass_utils import run_bass_kernel_spmd

# Problem constants (hardcoded per contest rules).
B, C, T = 16, 768, 1024
S = 12              # prediction steps
NNEG = 10
COPIES = NNEG + 1   # positive + negatives
CP = 12             # padded copies per timestep (extra dummy)
OFFSET = 1
NCORES = 8
NB = B // NCORES    # batches per core
CH = C // 128       # channel chunks
NQ = 4              # t-quarters
TQ = T // NQ        # 256
HIST = 13           # history columns kept per quarter (need t-1..t-12 plus 1 slack)
PW = TQ + HIST      # 269 = per-(chunk, step) block width in the proj buffer
GT_U = 32           # timesteps per gather tile
NGT = T // GT_U     # 32 gather tiles per batch
NI = GT_U * CP      # 384 gather indices per tile (must be % 128)
ST_U = 8            # timesteps per contrastive subtile
NSS = GT_U // ST_U  # 4 subtiles per gather tile
M_C = ST_U * CP     # 96 psum partitions per contrastive subtile
N_C = S * ST_U      # 96 moving columns per contrastive subtile

F16 = mybir.dt.float16
F32 = mybir.dt.float32
I16 = mybir.dt.int16

_CACHE = {}


def _build(repeat=1, do_proj=True, do_gather=True, do_contr=True):
    nc = bacc.Bacc(None, target_bir_lowering=False)
    x_d = nc.dram_tensor("x2", [NB, C, T], F16, kind="ExternalInput")
    w_d = nc.dram_tensor("w", [S, C, C], F16, kind="ExternalInput")
    yt_d = nc.dram_tensor("yt2", [NB, T, C], F16, kind="ExternalInput")
    idx_d = nc.dram_tensor("idx2", [NB, NGT, 128, NI // 16], I16,
                           kind="ExternalInput")
    praw_d = nc.dram_tensor("praw", [NB, NGT, M_C, NSS * N_C], F16,
                            kind="ExternalOutput")

    with tile.TileContext(nc) as tc:
        with (
            tc.tile_pool(name="wpool", bufs=1) as wpool,
            tc.tile_pool(name="xpool", bufs=1) as xpool,
            tc.tile_pool(name="projpool", bufs=1) as projpool,
            tc.tile_pool(name="gpool", bufs=3) as gpool,
            tc.tile_pool(name="idxpool", bufs=1) as idxpool,
            tc.tile_pool(name="stpool", bufs=3) as stpool,
            tc.tile_pool(name="ppsum", bufs=3, space="PSUM") as ppsum,
            tc.tile_pool(name="cpsum", bufs=3, space="PSUM") as cpsum,
        ):
            # Weights resident for the whole kernel: [p, s, chunk, d]
            w_sb = wpool.tile([128, S, CH, C], F16)
            nc.sync.dma_start(w_sb[:], w_d.rearrange("s (a p) d -> p s a d", p=128))

            def body(_i=None):
                _core_body(nc, tc, x_d, yt_d, idx_d, praw_d, w_sb,
                           xpool, projpool, gpool, idxpool, stpool,
                           ppsum, cpsum, do_proj, do_gather, do_contr)

            if repeat == 1:
                body()
            else:
                with tc.For_i(0, repeat, 1):
                    body()

    nc.compile()
    return nc


def _core_body(nc, tc, x_d, yt_d, idx_d, praw_d, w_sb,
               xpool, projpool, gpool, idxpool, stpool, ppsum, cpsum,
               do_proj=True, do_gather=True, do_contr=True):
    if True:
        if True:
            for b in range(NB):
                x_sb = xpool.tile([128, CH, T], F16)
                nc.sync.dma_start(x_sb[:],
                                  x_d[b].rearrange("(a p) t -> p a t", p=128))
                idx_sb = idxpool.tile([128, NGT, NI // 16], I16)
                nc.sync.dma_start(idx_sb[:],
                                  idx_d[b].rearrange("g p c -> p g c"))

                # proj quarter buffer: free layout (chunk, step, col),
                # col = HIST + (t - TQ*q); cols [0, HIST) hold history.
                pq = projpool.tile([128, CH, S, PW], F16)
                pq_ap = pq[:]
                PQF = CH * S * PW  # free elements per partition

                def pq_custom(extra_off, dims):
                    return bass.AP(pq_ap.tensor, pq_ap.offset + extra_off,
                                   [[PQF, 128]] + dims)

                # zero history pads (t < 0 for q=0); garbage there is
                # harmless numerically but keep sim/NaN clean.
                nc.any.memset(
                    pq_custom(0, [[S * PW, CH], [PW, S], [1, HIST]]), 0.0)

                for q in range(NQ):
                    if q > 0:
                        # carry last HIST columns into the history slots
                        nc.vector.tensor_copy(
                            pq_custom(0, [[S * PW, CH], [PW, S], [1, HIST]]),
                            pq_custom(TQ, [[S * PW, CH], [PW, S], [1, HIST]]),
                        )
                    # ---- projection for this quarter ----
                    for m in range(CH if do_proj else 0):
                        for sp in range(S // 2):
                            ps = ppsum.tile([128, 2 * TQ], F32)
                            for sh in range(2):
                                s = 2 * sp + sh
                                for k in range(CH):
                                    nc.tensor.matmul(
                                        ps[:, TQ * sh:TQ * (sh + 1)],
                                        w_sb[:, s, k, 128 * m:128 * (m + 1)],
                                        x_sb[:, k, TQ * q:TQ * (q + 1)],
                                        start=(k == 0), stop=(k == CH - 1),
                                    )
                            dst = pq_custom(
                                m * (S * PW) + (2 * sp) * PW + HIST,
                                [[PW, 2], [1, TQ]])
                            nc.vector.tensor_copy(
                                dst, ps[:].rearrange("p (a t) -> p a t", a=2))

                    # ---- contrastive logits for this quarter ----
                    for g in range(NGT // NQ):
                        gt = (NGT // NQ) * q + g
                        G = gpool.tile([128, CH, NI], F16)
                        if do_gather:
                            nc.gpsimd.dma_gather(
                                G[:], yt_d[b], idx_sb[:, gt, :],
                                NI, NI, C, transpose=True)
                        cps = cpsum.tile([128, NSS * N_C], F32)
                        for ss in range(NSS if do_contr else 0):
                            u_q = g * GT_U + ss * ST_U  # u offset in quarter
                            for k in range(CH):
                                rhs = pq_custom(
                                    k * (S * PW) + (HIST - 1) + u_q,
                                    [[PW - 1, S], [1, ST_U]])
                                nc.tensor.matmul(
                                    cps[0:M_C, N_C * ss:N_C * (ss + 1)],
                                    G[:, k, M_C * ss:M_C * (ss + 1)],
                                    rhs,
                                    start=(k == 0), stop=(k == CH - 1),
                                )
                        if do_contr:
                            stg = stpool.tile([M_C, NSS * N_C], F16)
                            nc.vector.tensor_copy(stg[:], cps[0:M_C, :])
                            nc.sync.dma_start(praw_d[b, gt], stg[:])


def _get_nc():
    if "nc" not in _CACHE:
        _CACHE["nc"] = _build()
    return _CACHE["nc"]


def _prep_inputs(x, y, W, neg_idxs):
    x16 = x.astype(np.float16)                                     # [B, C, T]
    w16 = np.ascontiguousarray(W.transpose(2, 0, 1).astype(np.float16))  # [S,C,C]
    yt16 = np.ascontiguousarray(np.swapaxes(y, 1, 2).astype(np.float16))  # [B,T,C]

    nl = (neg_idxs.astype(np.int64)
          - (np.arange(B, dtype=np.int64)[:, None] * T))           # [B, NNEG*T]
    nl = nl.reshape(B, NNEG, T)
    idxf = np.zeros((B, T, CP), np.int16)
    idxf[:, :, 0] = np.arange(T, dtype=np.int16)[None, :]
    idxf[:, :, 1:COPIES] = nl.transpose(0, 2, 1)
    # copy CP-1 stays 0 (dummy)
    flat = idxf.reshape(B, NGT, NI)
    wrap = flat.reshape(B, NGT, NI // 16, 16).transpose(0, 1, 3, 2)  # [B,NGT,16,NI/16]
    idx_sb = np.ascontiguousarray(np.tile(wrap, (1, 1, 8, 1)))       # [B,NGT,128,NI/16]
    return x16, w16, yt16, idx_sb, nl


def kernel(x, y, W, b, neg_idxs, _trace=False):
    x = np.asarray(x, np.float32)
    y = np.asarray(y, np.float32)
    W = np.asarray(W, np.float32)
    b = np.asarray(b, np.float32)
    neg_idxs = np.asarray(neg_idxs)

    nc = _get_nc()
    x16, w16, yt16, idx_sb, nl = _prep_inputs(x, y, W, neg_idxs)

    in_maps = []
    for ci in range(NCORES):
        sl = slice(NB * ci, NB * (ci + 1))
        in_maps.append({
            "x2": np.ascontiguousarray(x16[sl]),
            "w": w16,
            "yt2": np.ascontiguousarray(yt16[sl]),
            "idx2": np.ascontiguousarray(idx_sb[sl]),
        })

    res = run_bass_kernel_spmd(nc, in_maps, core_ids=list(range(NCORES)),
                               trace=_trace)
    _CACHE["last_result"] = res

    # ---- host-side diagonal extraction + assembly ----
    P = np.empty((B, T, CP, S), np.float32)
    ar = np.arange(ST_U)
    for ci in range(NCORES):
        pr = res.results[ci]["praw"].astype(np.float32)  # [NB, NGT, 96, 384]
        R = pr.reshape(NB, NGT, ST_U, CP, NSS, S, ST_U)
        # diagonal over u_loc (axes 2 and 6) -> [ST_U, NB, NGT, CP, NSS, S]
        D = R[:, :, ar, :, :, :, ar]
        # -> [NB, NGT, NSS, ST_U, CP, S] -> [NB, T, CP, S]
        P[NB * ci:NB * (ci + 1)] = (
            D.transpose(1, 2, 4, 0, 3, 5).reshape(NB, T, CP, S))

    # exact bias correction: preds += dot(bias, target_column)
    if np.any(b):
        dby = np.einsum("d,bdt->bt", b, y)               # [B, T]
        corr = np.empty((B, T, COPIES), np.float32)
        corr[:, :, 0] = dby
        bidx = np.arange(B)[:, None, None]
        corr[:, :, 1:] = dby[bidx, nl.transpose(0, 2, 1)]
        P[:, :, :COPIES, :] += corr[:, :, :, None]

    preds = []
    for i in range(S):
        off = i + OFFSET
        blk = P[:, off:T, :COPIES, i]          # [B, T-off, COPIES]
        preds.append(np.ascontiguousarray(blk.transpose(1, 0, 2)).reshape(-1, COPIES))
    predictions = np.concatenate(preds, axis=0)
    labels = np.zeros((predictions.shape[0],), np.int32)
    return predictions, labels
